# revision 12
# baseline (speedup 1.0000x reference)
"""Chamfer distance loss kernel for Trainium2 (Bass/Tile), 8-core SPMD.

Problem: pred_points, target_points [B=4, S=8, N=2048, D=3] fp32.
  d2[n,m] = |p_n|^2 + |t_m|^2 - 2 p_n.t_m  per (b,s) slice
  loss = sum_{slices, both directions} sum_points sqrt(max(min d2, 0)) / (2048*32)

Sharding: 32 independent (b,s) slices -> 4 per core (data parallel).

Device algorithm per slice, per direction:
  Augmented GEMM computing d2 directly (no cancellation):
    pred rows  [-2x,-2y,-2z, s_p, 1],  targ rows [x,y,z,1,s_t].
  Operands are split hi/lo into fp16 (x = h + l exactly up to ~2^-24), and
  the K dim packs all four h/l cross products:
    P blocks [Ph,Pl,Ph,Pl] x T blocks [Th,Th,Tl,Tl]  -> K=20, exact d2 in
  fp32 PSUM at bf16-rate (1 cyc/row) instead of fp32's 4.
  The 20-row operand sets are replicated at partition offsets 0/32/64/96 so
  four matmuls run concurrently in distinct PE row-groups (tile_position),
  filling two 2-bank PSUM span tiles (one pred chunk x all 2048 targets).
  Drain is split between ScalarE (copy to fp16 SBUF, then VectorE fp16
  min-folds at 2x + fused tensor_tensor_reduce) and direct VectorE fp32
  reduces, to balance engine load.
  Operand prep: squared norms + hi/lo splits in the natural [128,48]
  layout (full lane parallelism), PE-transpose, small gather DMAs.
Output: per-core [128, 8] partial sums of sqrt(d2min); host divides by
  2048*32 and sums across cores/partitions.
"""

import numpy as np

_B, _S, _N, _D = 4, 8, 2048, 3
_NCORES = 8
_SLICES_PER_CORE = (_B * _S) // _NCORES  # 4
_BIG = 1.0e30

_cached = {}


def _build_program(act_spans: int = 12, use_ttr: bool = False,
                   gps_fold: int = 0):
    """Build the SPMD Bass program (same program for all 8 cores).

    act_spans: of the 16 pred-chunk spans per slice-direction, how many are
      drained via ScalarE->fp16->VectorE (the rest: direct VectorE reduce).
    use_ttr: fuse the last fold + reduce into one tensor_tensor_reduce.
    gps_fold: route the first fp16 fold of this many ACT-spans to GpSimd.
    """
    import concourse.bass as bass
    import concourse.tile as tile
    from concourse import bacc, mybir

    f32 = mybir.dt.float32
    f16 = mybir.dt.float16
    AX = mybir.AxisListType.X
    OP = mybir.AluOpType

    nc = bacc.Bacc("TRN2", target_bir_lowering=False, debug=False,
                   num_devices=_NCORES)

    SL = _SLICES_PER_CORE
    predN = nc.dram_tensor("predN", [SL, 128, 48], f32, kind="ExternalInput").ap()
    targN = nc.dram_tensor("targN", [SL, 128, 48], f32, kind="ExternalInput").ap()
    ones16 = nc.dram_tensor("ones16", [1, _N], f16, kind="ExternalInput").ap()
    zeros16 = nc.dram_tensor("zeros16", [1, _N], f16, kind="ExternalInput").ap()
    zpad = nc.dram_tensor("zpad", [12, _N], f16, kind="ExternalInput").ap()
    identD = nc.dram_tensor("ident", [128, 128], f16, kind="ExternalInput").ap()
    out = nc.dram_tensor("sums", [128, 2 * SL], f32, kind="ExternalOutput").ap()

    NPC = _N // 128   # 16 pred chunks per slice-direction

    with tile.TileContext(nc) as tc:
        with (
            tc.tile_pool(name="const", bufs=1) as const_pool,
            tc.tile_pool(name="nat", bufs=2) as nat_pool,
            tc.tile_pool(name="aug", bufs=2) as aug_pool,
            tc.tile_pool(name="mins", bufs=2) as min_pool,
            tc.tile_pool(name="acc", bufs=1) as acc_pool,
            tc.tile_pool(name="span", bufs=3, space="PSUM") as span_pool,
            tc.tile_pool(name="prep", bufs=1, space="PSUM") as prep_pool,
            tc.tile_pool(name="f16", bufs=4) as f16_pool,
        ):
            ident = const_pool.tile([128, 128], f16)
            nc.sync.dma_start(ident[:], identD[:])
            sums_all = acc_pool.tile([128, 2 * SL], f32)

            for s in range(SL):
                # ---- natural-layout prep: norms, -2 scale, hi/lo splits ----
                pn = nat_pool.tile([128, 48], f32, tag="pn")
                tn = nat_pool.tile([128, 48], f32, tag="tn")
                nc.sync.dma_start(pn[:], predN[s])
                nc.sync.dma_start(tn[:], targN[s])
                psq = nat_pool.tile([128, 48], f32, tag="psq")
                tsq = nat_pool.tile([128, 48], f32, tag="tsq")
                nc.scalar.square(psq[:], pn[:])
                nc.scalar.square(tsq[:], tn[:])
                sp = nat_pool.tile([128, 16], f32, tag="sp")
                st = nat_pool.tile([128, 16], f32, tag="st")
                psq3 = psq[:].rearrange("p (i c) -> p i c", c=3)
                tsq3 = tsq[:].rearrange("p (i c) -> p i c", c=3)
                nc.vector.tensor_add(sp[:], psq3[:, :, 0], psq3[:, :, 1])
                nc.vector.tensor_add(sp[:], sp[:], psq3[:, :, 2])
                nc.vector.tensor_add(st[:], tsq3[:, :, 0], tsq3[:, :, 1])
                nc.vector.tensor_add(st[:], st[:], tsq3[:, :, 2])

                # packed [x(16)|y(16)|z(16)|s(16)] hi/lo fp16 tiles (c-major
                # so post-transpose gathers are contiguous partition slices);
                # -2 scale folded into the pred split ops.
                Ph = nat_pool.tile([128, 64], f16, tag="Ph")
                Pl = nat_pool.tile([128, 64], f16, tag="Pl")
                Th = nat_pool.tile([128, 64], f16, tag="Th")
                Tl = nat_pool.tile([128, 64], f16, tag="Tl")
                pn3 = pn[:].rearrange("p (i c) -> p i c", c=3)
                tn3 = tn[:].rearrange("p (i c) -> p i c", c=3)
                for c in range(3):
                    nc.vector.tensor_scalar_mul(
                        Ph[:, bass.ts(c, 16)], pn3[:, :, c], -2.0)
                    nc.vector.scalar_tensor_tensor(
                        Pl[:, bass.ts(c, 16)], pn3[:, :, c], -2.0,
                        Ph[:, bass.ts(c, 16)], op0=OP.mult, op1=OP.subtract)
                    nc.vector.tensor_copy(Th[:, bass.ts(c, 16)], tn3[:, :, c])
                    nc.vector.scalar_tensor_tensor(
                        Tl[:, bass.ts(c, 16)], tn3[:, :, c], 1.0,
                        Th[:, bass.ts(c, 16)], op0=OP.mult, op1=OP.subtract)
                nc.vector.tensor_copy(Ph[:, 48:64], sp[:])
                nc.vector.tensor_copy(Th[:, 48:64], st[:])
                nc.vector.scalar_tensor_tensor(
                    Pl[:, 48:64], sp[:], 1.0, Ph[:, 48:64],
                    op0=OP.mult, op1=OP.subtract)
                nc.vector.scalar_tensor_tensor(
                    Tl[:, 48:64], st[:], 1.0, Th[:, 48:64],
                    op0=OP.mult, op1=OP.subtract)

                # PE-transpose all four packs into one PSUM bank, then SBUF
                ps4 = prep_pool.tile([64, 512], f16, tag="ps4")
                for k, pk in enumerate((Ph, Pl, Th, Tl)):
                    nc.tensor.transpose(ps4[:, bass.ts(k, 128)], pk[:], ident[:])
                sb4 = nat_pool.tile([64, 512], f16, tag="sb4")
                nc.vector.tensor_copy(sb4[:], ps4[:])

                # ---- assemble augmented fp16 operand tiles ----
                # P blocks (at 32g+): [Ph(5) Pl(5) Ph(5) Pl(5)];
                # T blocks:           [Th(5) Th(5) Tl(5) Tl(5)]
                # row sets: pred [-2x,-2y,-2z, s_p, 1], targ [x,y,z,1,s_t]
                # point order j = i*128 + q  <-> natural point 16q + i
                P_aug = aug_pool.tile([128, _N], f16, tag="paug")
                T_aug = aug_pool.tile([128, _N], f16, tag="taug")

                def rowg(dst_rows, i=16):
                    return dst_rows.rearrange("a (i q) -> a i q", i=i)

                nc.sync.dma_start(rowg(P_aug[0:4, :]), sb4[0:64, 0:128])
                nc.sync.dma_start(rowg(P_aug[5:9, :]), sb4[0:64, 128:256])
                nc.sync.dma_start(rowg(T_aug[0:3, :]), sb4[0:48, 256:384])
                nc.sync.dma_start(rowg(T_aug[4:5, :]), sb4[48:64, 256:384])
                nc.sync.dma_start(rowg(T_aug[10:13, :]), sb4[0:48, 384:512])
                nc.sync.dma_start(rowg(T_aug[14:15, :]), sb4[48:64, 384:512])
                nc.sync.dma_start(P_aug[4:5, :], ones16[:])
                nc.sync.dma_start(P_aug[9:10, :], zeros16[:])
                nc.sync.dma_start(T_aug[3:4, :], ones16[:])
                nc.sync.dma_start(T_aug[13:14, :], zeros16[:])
                # duplicate h/l pairs to complete the 20-row blocks
                nc.sync.dma_start(P_aug[10:20, :], P_aug[0:10, :])
                nc.sync.dma_start(T_aug[5:10, :], T_aug[0:5, :])
                nc.sync.dma_start(T_aug[15:20, :], T_aug[10:15, :])
                # zero-pad rows 20:32 of each 32-row block (full row-groups
                # for tile_position; partial groups crash fp16 weight loads)
                for g in range(4):
                    nc.sync.dma_start(P_aug[32 * g + 20:32 * g + 32, :], zpad[:])
                    nc.sync.dma_start(T_aug[32 * g + 20:32 * g + 32, :], zpad[:])
                # replicate the 20-row block to partition offsets 32/64/96
                for g in range(1, 4):
                    nc.sync.dma_start(P_aug[32 * g:32 * g + 20, :], P_aug[0:20, :])
                    nc.sync.dma_start(T_aug[32 * g:32 * g + 20, :], T_aug[0:20, :])

                # ---- both directions: (weights, rhs) role swap ----
                for direction, (Wt, Rt) in enumerate(((P_aug, T_aug),
                                                      (T_aug, P_aug))):
                    dmw = min_pool.tile([128, 2 * NPC], f32, tag="dmw")
                    nc.vector.memset(dmw[:], _BIG)
                    slab = min_pool.tile([128, act_spans * 512], f16, tag="slab")
                    for pc in range(NPC):
                        spanA = span_pool.tile([128, 1024], f32, tag="span")
                        spanB = span_pool.tile([128, 1024], f32, tag="span")
                        spans = [spanA, spanB]
                        for g in range(4):
                            nc.tensor.matmul(
                                spans[g // 2][:, bass.ts(g % 2, 512)],
                                lhsT=Wt[32 * g:32 * g + 32, bass.ts(pc, 128)],
                                rhs=Rt[32 * g:32 * g + 32, bass.ts(g, 512)],
                                start=True, stop=True,
                                tile_position=(32 * g, 0),
                            )
                        if pc < act_spans:
                            t16 = f16_pool.tile([128, 2048], f16, tag="t16")
                            nc.scalar.copy(t16[:, 0:1024], spans[0][:])
                            nc.scalar.copy(t16[:, 1024:2048], spans[1][:])
                            h16 = f16_pool.tile([128, 1024], f16, tag="h16")
                            eng = nc.gpsimd if pc < gps_fold else nc.vector
                            eng.tensor_tensor(
                                h16[:], t16[:, 0:1024], t16[:, 1024:2048],
                                op=OP.min)
                            nc.vector.tensor_tensor(
                                slab[:, bass.ts(pc, 512)],
                                h16[:, 0:512], h16[:, 512:1024], op=OP.min)
                        else:
                            for j in range(2):
                                nc.vector.tensor_reduce(
                                    dmw[:, 2 * pc + j:2 * pc + j + 1],
                                    spans[j][:], axis=AX, op=OP.min)
                    # amortized tail over all ACT-routed spans:
                    # [128,A,512] -> [128,A,256] -> [128,A,128] -> [128,A]
                    sl3 = slab[:].rearrange("p (k f) -> p k f", f=512)
                    f3 = min_pool.tile([128, act_spans * 256], f16, tag="f3")
                    f3v = f3[:].rearrange("p (k f) -> p k f", f=256)
                    nc.vector.tensor_tensor(f3v[:], sl3[:, :, 0:256],
                                            sl3[:, :, 256:512], op=OP.min)
                    f4 = min_pool.tile([128, act_spans * 128], f16, tag="f4")
                    f4v = f4[:].rearrange("p (k f) -> p k f", f=128)
                    nc.vector.tensor_tensor(f4v[:], f3v[:, :, 0:128],
                                            f3v[:, :, 128:256], op=OP.min)
                    nc.vector.tensor_reduce(
                        dmw[:, 0:2 * act_spans:2], f4v[:], axis=AX, op=OP.min)
                    dmin = min_pool.tile([128, NPC], f32, tag="dmin")
                    dmw2 = dmw[:].rearrange("p (i two) -> p i two", two=2)
                    nc.vector.tensor_tensor(dmin[:], dmw2[:, :, 0],
                                            dmw2[:, :, 1], op=OP.min)
                    nc.vector.tensor_scalar_max(dmin[:], dmin[:], 0.0)
                    nc.scalar.sqrt(dmin[:], dmin[:])
                    nc.vector.reduce_sum(
                        sums_all[:, 2 * s + direction: 2 * s + direction + 1],
                        dmin[:], axis=AX)

            nc.sync.dma_start(out[:], sums_all[:])

    nc.compile()
    return nc


def _get_program():
    key = "prog"
    if key not in _cached:
        _cached[key] = _build_program()
    return _cached[key]


def _shard_inputs(pred_points: np.ndarray, target_points: np.ndarray):
    """Slice the 32 (b,s) slices into 8 groups of 4, natural layout only."""
    pred = np.ascontiguousarray(pred_points, dtype=np.float32).reshape(
        _B * _S, _N, _D)
    targ = np.ascontiguousarray(target_points, dtype=np.float32).reshape(
        _B * _S, _N, _D)
    ones16 = np.ones((1, _N), dtype=np.float16)
    zeros16 = np.zeros((1, _N), dtype=np.float16)
    zpad = np.zeros((12, _N), dtype=np.float16)
    ident = np.eye(128, dtype=np.float16)
    in_maps = []
    for c in range(_NCORES):
        sl = slice(c * _SLICES_PER_CORE, (c + 1) * _SLICES_PER_CORE)
        in_maps.append({
            "predN": np.ascontiguousarray(
                pred[sl].reshape(_SLICES_PER_CORE, 128, 48)),
            "targN": np.ascontiguousarray(
                targ[sl].reshape(_SLICES_PER_CORE, 128, 48)),
            "ones16": ones16,
            "zeros16": zeros16,
            "zpad": zpad,
            "ident": ident,
        })
    return in_maps


def _run(in_maps, trace=False, tmpdir=None):
    from concourse.bass_utils import run_bass_kernel_spmd
    nc = _get_program()
    return run_bass_kernel_spmd(nc, in_maps, list(range(_NCORES)),
                                trace=trace, tmpdir=tmpdir)


def kernel(pred_points: np.ndarray, target_points: np.ndarray) -> np.ndarray:
    in_maps = _shard_inputs(pred_points, target_points)
    res = _run(in_maps)
    total = np.float64(0.0)
    for c in range(_NCORES):
        total += np.float64(res.results[c]["sums"].sum(dtype=np.float64))
    return np.float32(total / (_N * _B * _S))


# revision 13
# speedup vs baseline: 1.1028x; 1.1028x over previous
"""Chamfer distance loss kernel for Trainium2 (Bass/Tile), 8-core SPMD.

Problem: pred_points, target_points [B=4, S=8, N=2048, D=3] fp32.
  d2[n,m] = |p_n|^2 + |t_m|^2 - 2 p_n.t_m  per (b,s) slice
  loss = sum_{slices, both directions} sum_points sqrt(max(min d2, 0)) / (2048*32)

Sharding: 32 independent (b,s) slices -> 4 per core (data parallel).

Device algorithm per slice, per direction:
  Augmented GEMM computing d2 directly (no cancellation):
    pred rows  [-2x,-2y,-2z, s_p, 1],  targ rows [x,y,z,1,s_t].
  Operands are split hi/lo into fp16 (x = h + l exactly up to ~2^-24), and
  the K dim packs all four h/l cross products:
    P blocks [Ph,Pl,Ph,Pl] x T blocks [Th,Th,Tl,Tl]  -> K=20, exact d2 in
  fp32 PSUM at bf16-rate (1 cyc/row) instead of fp32's 4.
  The 20-row operand sets are replicated at partition offsets 0/32/64/96 so
  four matmuls run concurrently in distinct PE row-groups (tile_position),
  filling two 2-bank PSUM span tiles (one pred chunk x all 2048 targets).
  Drain is split between ScalarE (copy to fp16 SBUF, then VectorE fp16
  min-folds at 2x + fused tensor_tensor_reduce) and direct VectorE fp32
  reduces, to balance engine load.
  Operand prep: squared norms + hi/lo splits in the natural [128,48]
  layout (full lane parallelism), PE-transpose, small gather DMAs.
Output: per-core [128, 8] partial sums of sqrt(d2min); host divides by
  2048*32 and sums across cores/partitions.
"""

import numpy as np

_B, _S, _N, _D = 4, 8, 2048, 3
_NCORES = 8
_SLICES_PER_CORE = (_B * _S) // _NCORES  # 4
_BIG = 1.0e30

_cached = {}


def _build_program(act_spans: int = 12, use_ttr: bool = False,
                   gps_fold: int = 0):
    """Build the SPMD Bass program (same program for all 8 cores).

    act_spans: of the 16 pred-chunk spans per slice-direction, how many are
      drained via ScalarE->fp16->VectorE (the rest: direct VectorE reduce).
    use_ttr: fuse the last fold + reduce into one tensor_tensor_reduce.
    gps_fold: route the first fp16 fold of this many ACT-spans to GpSimd.
    """
    import concourse.bass as bass
    import concourse.tile as tile
    from concourse import bacc, mybir

    f32 = mybir.dt.float32
    f16 = mybir.dt.float16
    AX = mybir.AxisListType.X
    OP = mybir.AluOpType

    nc = bacc.Bacc("TRN2", target_bir_lowering=False, debug=False,
                   num_devices=_NCORES)

    SL = _SLICES_PER_CORE
    predN = nc.dram_tensor("predN", [SL, 128, 48], f32, kind="ExternalInput").ap()
    targN = nc.dram_tensor("targN", [SL, 128, 48], f32, kind="ExternalInput").ap()
    ones16 = nc.dram_tensor("ones16", [1, _N], f16, kind="ExternalInput").ap()
    zeros16 = nc.dram_tensor("zeros16", [1, _N], f16, kind="ExternalInput").ap()
    zpad = nc.dram_tensor("zpad", [12, _N], f16, kind="ExternalInput").ap()
    identD = nc.dram_tensor("ident", [128, 128], f16, kind="ExternalInput").ap()
    out = nc.dram_tensor("sums", [128, 2 * SL], f32, kind="ExternalOutput").ap()

    NPC = _N // 128   # 16 pred chunks per slice-direction

    with tile.TileContext(nc) as tc:
        with (
            tc.tile_pool(name="const", bufs=1) as const_pool,
            tc.tile_pool(name="nat", bufs=2) as nat_pool,
            tc.tile_pool(name="aug", bufs=2) as aug_pool,
            tc.tile_pool(name="mins", bufs=2) as min_pool,
            tc.tile_pool(name="acc", bufs=1) as acc_pool,
            tc.tile_pool(name="span", bufs=3, space="PSUM") as span_pool,
            tc.tile_pool(name="prep", bufs=1, space="PSUM") as prep_pool,
            tc.tile_pool(name="f16", bufs=4) as f16_pool,
        ):
            ident = const_pool.tile([128, 128], f16)
            nc.sync.dma_start(ident[:], identD[:])
            sums_all = acc_pool.tile([128, 2 * SL], f32)

            for s in range(SL):
                # ---- natural-layout prep: norms, -2 scale, hi/lo splits ----
                pn = nat_pool.tile([128, 48], f32, tag="pn")
                tn = nat_pool.tile([128, 48], f32, tag="tn")
                nc.sync.dma_start(pn[:], predN[s])
                nc.sync.dma_start(tn[:], targN[s])
                psq = nat_pool.tile([128, 48], f32, tag="psq")
                tsq = nat_pool.tile([128, 48], f32, tag="tsq")
                nc.scalar.square(psq[:], pn[:])
                nc.scalar.square(tsq[:], tn[:])
                sp = nat_pool.tile([128, 16], f32, tag="sp")
                st = nat_pool.tile([128, 16], f32, tag="st")
                psq3 = psq[:].rearrange("p (i c) -> p i c", c=3)
                tsq3 = tsq[:].rearrange("p (i c) -> p i c", c=3)
                nc.vector.tensor_add(sp[:], psq3[:, :, 0], psq3[:, :, 1])
                nc.vector.tensor_add(sp[:], sp[:], psq3[:, :, 2])
                nc.vector.tensor_add(st[:], tsq3[:, :, 0], tsq3[:, :, 1])
                nc.vector.tensor_add(st[:], st[:], tsq3[:, :, 2])

                # packed [x(16)|y(16)|z(16)|s(16)] hi/lo fp16 tiles (c-major
                # so post-transpose gathers are contiguous partition slices);
                # -2 scale folded into the pred split ops.
                Ph = nat_pool.tile([128, 64], f16, tag="Ph")
                Pl = nat_pool.tile([128, 64], f16, tag="Pl")
                Th = nat_pool.tile([128, 64], f16, tag="Th")
                Tl = nat_pool.tile([128, 64], f16, tag="Tl")
                pn3 = pn[:].rearrange("p (i c) -> p i c", c=3)
                tn3 = tn[:].rearrange("p (i c) -> p i c", c=3)
                for c in range(3):
                    nc.vector.tensor_scalar_mul(
                        Ph[:, bass.ts(c, 16)], pn3[:, :, c], -2.0)
                    nc.vector.scalar_tensor_tensor(
                        Pl[:, bass.ts(c, 16)], pn3[:, :, c], -2.0,
                        Ph[:, bass.ts(c, 16)], op0=OP.mult, op1=OP.subtract)
                    nc.vector.tensor_copy(Th[:, bass.ts(c, 16)], tn3[:, :, c])
                    nc.vector.scalar_tensor_tensor(
                        Tl[:, bass.ts(c, 16)], tn3[:, :, c], 1.0,
                        Th[:, bass.ts(c, 16)], op0=OP.mult, op1=OP.subtract)
                nc.vector.tensor_copy(Ph[:, 48:64], sp[:])
                nc.vector.tensor_copy(Th[:, 48:64], st[:])
                nc.vector.scalar_tensor_tensor(
                    Pl[:, 48:64], sp[:], 1.0, Ph[:, 48:64],
                    op0=OP.mult, op1=OP.subtract)
                nc.vector.scalar_tensor_tensor(
                    Tl[:, 48:64], st[:], 1.0, Th[:, 48:64],
                    op0=OP.mult, op1=OP.subtract)

                # PE-transpose all four packs into one PSUM bank, then SBUF
                ps4 = prep_pool.tile([64, 512], f16, tag="ps4")
                for k, pk in enumerate((Ph, Pl, Th, Tl)):
                    nc.tensor.transpose(ps4[:, bass.ts(k, 128)], pk[:], ident[:])
                sb4 = nat_pool.tile([64, 512], f16, tag="sb4")
                nc.vector.tensor_copy(sb4[:], ps4[:])

                # ---- assemble augmented fp16 operand tiles ----
                # P blocks (at 32g+): [Ph(5) Pl(5) Ph(5) Pl(5)];
                # T blocks:           [Th(5) Th(5) Tl(5) Tl(5)]
                # row sets: pred [-2x,-2y,-2z, s_p, 1], targ [x,y,z,1,s_t]
                # point order j = i*128 + q  <-> natural point 16q + i
                P_aug = aug_pool.tile([128, _N], f16, tag="paug")
                T_aug = aug_pool.tile([128, _N], f16, tag="taug")

                def rowg(dst_rows, i=16):
                    return dst_rows.rearrange("a (i q) -> a i q", i=i)

                nc.sync.dma_start(rowg(P_aug[0:4, :]), sb4[0:64, 0:128])
                nc.sync.dma_start(rowg(P_aug[5:9, :]), sb4[0:64, 128:256])
                nc.sync.dma_start(rowg(T_aug[0:3, :]), sb4[0:48, 256:384])
                nc.sync.dma_start(rowg(T_aug[4:5, :]), sb4[48:64, 256:384])
                nc.sync.dma_start(rowg(T_aug[10:13, :]), sb4[0:48, 384:512])
                nc.sync.dma_start(rowg(T_aug[14:15, :]), sb4[48:64, 384:512])
                nc.sync.dma_start(P_aug[4:5, :], ones16[:])
                nc.sync.dma_start(P_aug[9:10, :], zeros16[:])
                nc.sync.dma_start(T_aug[3:4, :], ones16[:])
                nc.sync.dma_start(T_aug[13:14, :], zeros16[:])
                # duplicate h/l pairs to complete the 20-row blocks
                nc.sync.dma_start(P_aug[10:20, :], P_aug[0:10, :])
                nc.sync.dma_start(T_aug[5:10, :], T_aug[0:5, :])
                nc.sync.dma_start(T_aug[15:20, :], T_aug[10:15, :])
                # zero-pad rows 20:32 of each 32-row block (full row-groups
                # for tile_position; partial groups crash fp16 weight loads)
                for g in range(4):
                    nc.sync.dma_start(P_aug[32 * g + 20:32 * g + 32, :], zpad[:])
                    nc.sync.dma_start(T_aug[32 * g + 20:32 * g + 32, :], zpad[:])
                # replicate the 20-row block to partition offsets 32/64/96
                for g in range(1, 4):
                    nc.sync.dma_start(P_aug[32 * g:32 * g + 20, :], P_aug[0:20, :])
                    nc.sync.dma_start(T_aug[32 * g:32 * g + 20, :], T_aug[0:20, :])

                # ---- both directions: (weights, rhs) role swap ----
                for direction, (Wt, Rt) in enumerate(((P_aug, T_aug),
                                                      (T_aug, P_aug))):
                    dmw = min_pool.tile([128, 2 * NPC], f32, tag="dmw")
                    nc.vector.memset(dmw[:], _BIG)
                    slab = min_pool.tile([128, NPC * 512], f16, tag="slab")
                    for pc in range(NPC):
                        spanA = span_pool.tile([128, 1024], f32, tag="span")
                        spanB = span_pool.tile([128, 1024], f32, tag="span")
                        spans = [spanA, spanB]
                        for g in range(4):
                            nc.tensor.matmul(
                                spans[g // 2][:, bass.ts(g % 2, 512)],
                                lhsT=Wt[32 * g:32 * g + 32, bass.ts(pc, 128)],
                                rhs=Rt[32 * g:32 * g + 32, bass.ts(g, 512)],
                                start=True, stop=True,
                                tile_position=(32 * g, 0),
                            )
                        # balanced within-span split: ScalarE casts 3/4 of
                        # the span to fp16, VectorE direct-reduces the rest
                        # and min-folds the fp16 part into the slab.
                        t16 = f16_pool.tile([128, 1536], f16, tag="t16")
                        nc.scalar.copy(t16[:, 0:1024], spans[0][:])
                        nc.scalar.copy(t16[:, 1024:1536], spans[1][:, 0:512])
                        nc.vector.tensor_reduce(
                            dmw[:, 2 * pc + 1:2 * pc + 2],
                            spans[1][:, 512:1024], axis=AX, op=OP.min)
                        u16 = f16_pool.tile([128, 512], f16, tag="u16")
                        nc.vector.tensor_tensor(
                            u16[:], t16[:, 0:512], t16[:, 512:1024], op=OP.min)
                        nc.vector.tensor_tensor(
                            slab[:, bass.ts(pc, 512)],
                            u16[:], t16[:, 1024:1536], op=OP.min)
                    # amortized tail over all spans:
                    # [128,16,512] -> [128,16,256] -> [128,16,128] -> [128,16]
                    sl3 = slab[:].rearrange("p (k f) -> p k f", f=512)
                    f3 = min_pool.tile([128, NPC * 256], f16, tag="f3")
                    f3v = f3[:].rearrange("p (k f) -> p k f", f=256)
                    nc.vector.tensor_tensor(f3v[:], sl3[:, :, 0:256],
                                            sl3[:, :, 256:512], op=OP.min)
                    f4 = min_pool.tile([128, NPC * 128], f16, tag="f4")
                    f4v = f4[:].rearrange("p (k f) -> p k f", f=128)
                    nc.vector.tensor_tensor(f4v[:], f3v[:, :, 0:128],
                                            f3v[:, :, 128:256], op=OP.min)
                    nc.vector.tensor_reduce(
                        dmw[:, 0:2 * NPC:2], f4v[:], axis=AX, op=OP.min)
                    dmin = min_pool.tile([128, NPC], f32, tag="dmin")
                    dmw2 = dmw[:].rearrange("p (i two) -> p i two", two=2)
                    nc.vector.tensor_tensor(dmin[:], dmw2[:, :, 0],
                                            dmw2[:, :, 1], op=OP.min)
                    nc.vector.tensor_scalar_max(dmin[:], dmin[:], 0.0)
                    nc.scalar.sqrt(dmin[:], dmin[:])
                    nc.vector.reduce_sum(
                        sums_all[:, 2 * s + direction: 2 * s + direction + 1],
                        dmin[:], axis=AX)

            nc.sync.dma_start(out[:], sums_all[:])

    nc.compile()
    return nc


def _get_program():
    key = "prog"
    if key not in _cached:
        _cached[key] = _build_program()
    return _cached[key]


def _shard_inputs(pred_points: np.ndarray, target_points: np.ndarray):
    """Slice the 32 (b,s) slices into 8 groups of 4, natural layout only."""
    pred = np.ascontiguousarray(pred_points, dtype=np.float32).reshape(
        _B * _S, _N, _D)
    targ = np.ascontiguousarray(target_points, dtype=np.float32).reshape(
        _B * _S, _N, _D)
    ones16 = np.ones((1, _N), dtype=np.float16)
    zeros16 = np.zeros((1, _N), dtype=np.float16)
    zpad = np.zeros((12, _N), dtype=np.float16)
    ident = np.eye(128, dtype=np.float16)
    in_maps = []
    for c in range(_NCORES):
        sl = slice(c * _SLICES_PER_CORE, (c + 1) * _SLICES_PER_CORE)
        in_maps.append({
            "predN": np.ascontiguousarray(
                pred[sl].reshape(_SLICES_PER_CORE, 128, 48)),
            "targN": np.ascontiguousarray(
                targ[sl].reshape(_SLICES_PER_CORE, 128, 48)),
            "ones16": ones16,
            "zeros16": zeros16,
            "zpad": zpad,
            "ident": ident,
        })
    return in_maps


def _run(in_maps, trace=False, tmpdir=None):
    from concourse.bass_utils import run_bass_kernel_spmd
    nc = _get_program()
    return run_bass_kernel_spmd(nc, in_maps, list(range(_NCORES)),
                                trace=trace, tmpdir=tmpdir)


def kernel(pred_points: np.ndarray, target_points: np.ndarray) -> np.ndarray:
    in_maps = _shard_inputs(pred_points, target_points)
    res = _run(in_maps)
    total = np.float64(0.0)
    for c in range(_NCORES):
        total += np.float64(res.results[c]["sums"].sum(dtype=np.float64))
    return np.float32(total / (_N * _B * _S))
